# revision 15
# baseline (speedup 1.0000x reference)
"""GCN layer kernel for 8 Trainium2 NeuronCores.

Math (reference):
    h  = (x @ W1.T + b1) @ W2.T + b2
    A  = adj + I
    r  = rowsum(A) ** -0.5
    out = (r[:,None] * A * r[None,:]) @ h
        = r ⊙ (adj @ (r ⊙ h)) + r ⊙ (r ⊙ h)

Associativity: with h1 = x @ W1.T + b1 and g1 = [r ⊙ h1 | r] (257 cols),
    acc  = adj @ g1 + g1_own            # [rows, 257]  (spmm, K=8192)
    out  = r ⊙ (acc[:, :256] @ W2.T + acc[:, 256] ⊗ b2)

Distribution: row-shard adj and x across 8 cores (1024 rows each).
Per core:
  - stream the 32MB fp32 adj shard once; one ACT op per chunk does the
    bf16 cast + rowsum (accum_out); XBAR DMA-transposes (SBUF->SBUF, on
    the ACT HWDGE ring) land bf16 adjT [128, 64, 1024] resident in SBUF.
  - mm1 on the local x shard -> h1 (bf16, hid on partitions); AllGather
    h1 (1MB, overlaps the adj streaming).
  - AllGather rowsums (4KB); r = sqrt(1/(rowsum+1)).
  - g1 built by DMA-transposing gathered h1 + r scale.
  - spmm: per local i-tile accumulate 64 bf16 matmuls [128x128]@[128x257]
    into one PSUM bank; small @W2T tail + bias + r_i scale; DMA out.
"""

import sys

if "/opt/trn_rl_repo" not in sys.path:
    sys.path.insert(0, "/opt/trn_rl_repo")

import numpy as np

import concourse.bass as bass
import concourse.mybir as mybir
import concourse.tile as tile
from concourse import bacc
from concourse.bass_utils import run_bass_kernel_spmd
from concourse.masks import make_identity

F32 = mybir.dt.float32
BF16 = mybir.dt.bfloat16
P = 128
NCORES = 8


def build_nc(n_nodes=8192, in_f=512, hid=256, out_f=512, cw=2048, debug_dump=False):
    """Build and compile the per-core SPMD kernel."""
    cw = min(cw, n_nodes)
    SH = n_nodes // NCORES      # shard rows per core
    KB = SH // P                # row blocks in shard (= local i tiles)
    JT = n_nodes // P           # global j tiles
    QN = n_nodes // cw          # adj chunks per row block
    BPC = cw // P               # 128-blocks per chunk
    FT = in_f // P              # input feature tiles
    HT = hid // P               # hidden tiles
    OF = out_f
    GW = hid + 1                # g1 logical width: [r*h1 | r]
    GWP = hid + 16              # padded block stride (32B-aligned for tDMA dests)
    IG = min(4, KB)             # i-stripes per mm1 group
    NG = IG * P                 # mm1 rhs free size

    nc = bacc.Bacc(
        "TRN2",
        target_bir_lowering=False,
        debug=False,
        num_devices=NCORES,
        dynamic_dma_scratch_size=4096,
    )
    adj_s = nc.dram_tensor("adj_shard", [SH, n_nodes], F32, kind="ExternalInput")
    x_s = nc.dram_tensor("x_shard", [SH, in_f], F32, kind="ExternalInput")
    W1 = nc.dram_tensor("W1", [hid, in_f], F32, kind="ExternalInput")
    b1 = nc.dram_tensor("b1", [hid], F32, kind="ExternalInput")
    W2 = nc.dram_tensor("W2", [out_f, hid], F32, kind="ExternalInput")
    b2 = nc.dram_tensor("b2", [out_f], F32, kind="ExternalInput")
    out = nc.dram_tensor("out_shard", [SH, OF], F32, kind="ExternalOutput")

    with tile.TileContext(nc) as tc:
        with (
            tc.tile_pool(name="const", bufs=1) as cpool,
            tc.tile_pool(name="big", bufs=1) as bigpool,
            tc.tile_pool(name="adj_in", bufs=2) as adj_in_pool,
            tc.tile_pool(name="chbf", bufs=3) as chbf_pool,
            tc.tile_pool(name="xt", bufs=1) as xt_pool,
            tc.tile_pool(name="acc_bf", bufs=2) as accbf_pool,
            tc.tile_pool(name="vb", bufs=1) as vb_pool,
            tc.tile_pool(name="out_sb", bufs=2) as out_pool,
            tc.tile_pool(name="pmm", bufs=2, space="PSUM") as pmm_pool,
            tc.tile_pool(name="pacc", bufs=2, space="PSUM") as pacc_pool,
            tc.tile_pool(name="ptail", bufs=2, space="PSUM") as ptail_pool,
            tc.tile_pool(name="dram", bufs=1, space="DRAM") as dram,
        ):
            # ---------- constants ----------
            ones_bf = cpool.tile([1, max(OF, P)], BF16)
            nc.vector.memset(ones_bf, 1.0)
            # SWDGE DMA casts f32 -> bf16 in flight
            b1_bf = cpool.tile([1, hid], BF16)
            nc.gpsimd.dma_start(b1_bf, b1.ap()[None, :])
            b2_bf = cpool.tile([1, out_f], BF16)
            nc.gpsimd.dma_start(b2_bf, b2.ap()[None, :])
            # b2 broadcast to all partitions (for the v*b2 outer product)
            pb = pmm_pool.tile([P, OF], F32, tag="pmm")
            nc.tensor.matmul(pb, ones_bf[:1, :P], b2_bf[:1, :], start=True, stop=True)
            b2_bcast = cpool.tile([P, OF], BF16)
            nc.scalar.copy(b2_bcast, pb)

            # ---------- weights: W1T [P, FT, hid], W2T [P, HT, OF] via tDMA ----------
            W1T = cpool.tile([P, FT, hid], BF16)
            W2T = cpool.tile([P, HT, OF], BF16)
            for s in range(HT):  # W1 row stripes (hid)
                w_in = adj_in_pool.tile([P, cw], F32, tag="adj_in")
                nc.sync.dma_start(w_in[:, :in_f], W1.ap()[s * P:(s + 1) * P, :])
                w_bf = chbf_pool.tile([P, cw], BF16, tag="chbf")
                nc.scalar.copy(w_bf[:, :in_f], w_in[:, :in_f])
                for ft in range(FT):
                    nc.scalar.dma_start_transpose(
                        W1T[:, ft, s * P:(s + 1) * P], w_bf[:, ft * P:(ft + 1) * P]
                    )
            for s in range(out_f // P):  # W2 row stripes (out_f)
                w_in = adj_in_pool.tile([P, cw], F32, tag="adj_in")
                nc.sync.dma_start(w_in[:, :hid], W2.ap()[s * P:(s + 1) * P, :])
                w_bf = chbf_pool.tile([P, cw], BF16, tag="chbf")
                nc.scalar.copy(w_bf[:, :hid], w_in[:, :hid])
                for ht in range(HT):
                    nc.scalar.dma_start_transpose(
                        W2T[:, ht, s * P:(s + 1) * P], w_bf[:, ht * P:(ht + 1) * P]
                    )

            # ---------- mm1: h1_c [P, HT, SH] bf16 (hid on partitions) ----------
            h1_c = bigpool.tile([P, HT, SH], BF16)
            for grp in range(SH // NG):
                xt = xt_pool.tile([P, FT, NG], BF16)
                for s in range(IG):
                    x_in = adj_in_pool.tile([P, cw], F32, tag="adj_in")
                    row0 = (grp * IG + s) * P
                    nc.sync.dma_start(x_in[:, :in_f], x_s.ap()[row0:row0 + P, :])
                    x_bf = chbf_pool.tile([P, cw], BF16, tag="chbf")
                    nc.scalar.copy(x_bf[:, :in_f], x_in[:, :in_f])
                    for ft in range(FT):
                        nc.scalar.dma_start_transpose(
                            xt[:, ft, s * P:(s + 1) * P],
                            x_bf[:, ft * P:(ft + 1) * P],
                        )
                for ht in range(HT):
                    pm = pmm_pool.tile([P, OF], F32, tag="pmm")
                    pm1 = pm[:, :NG]
                    nc.tensor.matmul(
                        pm1, b1_bf[:1, ht * P:(ht + 1) * P], ones_bf[:1, :NG],
                        start=True, stop=False,
                    )
                    for ft in range(FT):
                        nc.tensor.matmul(
                            pm1,
                            W1T[:, ft, ht * P:(ht + 1) * P],
                            xt[:, ft, :],
                            start=False, stop=(ft == FT - 1),
                        )
                    nc.scalar.copy(h1_c[:, ht, grp * NG:(grp + 1) * NG], pm1)

            # ---------- AllGather h1 ----------
            h1d = dram.tile([HT, P, SH], BF16)
            for ht in range(HT):
                nc.sync.dma_start(h1d[ht], h1_c[:, ht, :])
            h1g = dram.tile([NCORES * HT, P, SH], BF16, addr_space="Shared")
            nc.gpsimd.collective_compute(
                "AllGather",
                mybir.AluOpType.bypass,
                replica_groups=[list(range(NCORES))],
                ins=[h1d.opt()],
                outs=[h1g.opt()],
            )

            # ---------- stream adj shard: fused cast+rowsum, tDMA into adjT ----------
            adjT = bigpool.tile([P, JT, SH], BF16)
            rowsum_c = cpool.tile([P, KB], F32)
            for k in range(KB):
                partials = cpool.tile([P, QN], F32, tag="partials")
                for q in range(QN):
                    ch = adj_in_pool.tile([P, cw], F32, tag="adj_in")
                    nc.sync.dma_start(
                        ch, adj_s.ap()[k * P:(k + 1) * P, q * cw:(q + 1) * cw]
                    )
                    chbf = chbf_pool.tile([P, cw], BF16, tag="chbf")
                    nc.scalar.activation(
                        chbf, ch, mybir.ActivationFunctionType.Identity,
                        bias=0.0, scale=1.0, accum_out=partials[:, q:q + 1],
                    )
                    for b in range(BPC):
                        nc.scalar.dma_start_transpose(
                            adjT[:, q * BPC + b, k * P:(k + 1) * P],
                            chbf[:, b * P:(b + 1) * P],
                        )
                nc.vector.reduce_sum(
                    rowsum_c[:, k:k + 1], partials, axis=mybir.AxisListType.X
                )

            # ---------- AllGather rowsums; r vectors ----------
            rsd = dram.tile([P, KB], F32)
            nc.sync.dma_start(rsd, rowsum_c)
            rsg = dram.tile([NCORES * P, KB], F32, addr_space="Shared")
            nc.gpsimd.collective_compute(
                "AllGather",
                mybir.AluOpType.bypass,
                replica_groups=[list(range(NCORES))],
                ins=[rsd.opt()],
                outs=[rsg.opt()],
            )
            # r_sb [P, NCORES, KB]: r for global tile jt = (c, k) at [:, c, k]
            rs_t = cpool.tile([P, NCORES, KB], F32)
            nc.sync.dma_start(rs_t, rsg.rearrange("(c p) k -> p c k", p=P))
            r_sb = cpool.tile([P, NCORES, KB], F32)
            nc.vector.tensor_scalar_add(rs_t, rs_t, 1.0)
            nc.vector.reciprocal(rs_t, rs_t)
            nc.scalar.sqrt(r_sb, rs_t)
            # local r for own rows
            r_own = cpool.tile([P, KB], F32)
            ro_t = cpool.tile([P, KB], F32)
            nc.vector.tensor_scalar_add(ro_t, rowsum_c, 1.0)
            nc.vector.reciprocal(ro_t, ro_t)
            nc.scalar.sqrt(r_own, ro_t)

            # ---------- g1 [P, JT, GW] = [r ⊙ h1ᵀ | r] for all rows ----------
            g1 = bigpool.tile([P, JT, GWP], BF16)
            for jt in range(JT):
                c, k = jt // KB, jt % KB
                for ht in range(HT):
                    nc.scalar.dma_start_transpose(
                        g1[:, jt, ht * P:(ht + 1) * P],
                        h1g[c * HT + ht, :, k * P:(k + 1) * P],
                    )
                rc = r_sb[:, c, k:k + 1]
                nc.vector.tensor_scalar_mul(g1[:, jt, :hid], g1[:, jt, :hid], rc)
                nc.vector.tensor_copy(g1[:, jt, hid:GW], rc)
            # own-row g1 from local h1 (identity term)
            g1own = bigpool.tile([P, KB, GWP], BF16)
            for k in range(KB):
                for ht in range(HT):
                    nc.scalar.dma_start_transpose(
                        g1own[:, k, ht * P:(ht + 1) * P],
                        h1_c[:, ht, k * P:(k + 1) * P],
                    )
                rc = r_own[:, k:k + 1]
                nc.vector.tensor_scalar_mul(g1own[:, k, :hid], g1own[:, k, :hid], rc)
                nc.vector.tensor_copy(g1own[:, k, hid:GW], rc)

            # ---------- debug dumps ----------
            if debug_dump:
                d_adjT = nc.dram_tensor(
                    "d_adjT", [P, JT, SH], BF16, kind="ExternalOutput"
                )
                nc.sync.dma_start(d_adjT.ap(), adjT)
                d_g1 = nc.dram_tensor("d_g1", [P, JT, GWP], BF16, kind="ExternalOutput")
                nc.sync.dma_start(d_g1.ap(), g1)
                d_h1 = nc.dram_tensor("d_h1", [P, HT, SH], BF16, kind="ExternalOutput")
                nc.sync.dma_start(d_h1.ap(), h1_c)
                d_rs = nc.dram_tensor("d_rs", [P, KB], F32, kind="ExternalOutput")
                nc.sync.dma_start(d_rs.ap(), rowsum_c)

            # ---------- spmm + tail ----------
            for it in range(KB):
                acc = pacc_pool.tile([P, GW], F32)
                for jt in range(JT):
                    nc.tensor.matmul(
                        acc,
                        adjT[:, jt, it * P:(it + 1) * P],
                        g1[:, jt, :GW],
                        start=(jt == 0),
                        stop=(jt == JT - 1),
                    )
                # acc += g1_own (identity term), cast to bf16 for the tail
                nc.vector.tensor_tensor(acc, acc, g1own[:, it, :GW], mybir.AluOpType.add)
                accbf = accbf_pool.tile([P, hid], BF16)
                nc.scalar.copy(accbf, acc[:, :hid])
                vcol = accbf_pool.tile([P, 1], F32, tag="vcol")
                nc.vector.tensor_copy(vcol, acc[:, hid:GW])
                # accT [P(h1), HT, P(i)] via SBUF->SBUF tDMA
                accT = accbf_pool.tile([P, HT, P], BF16, tag="accT")
                for ht in range(HT):
                    nc.scalar.dma_start_transpose(
                        accT[:, ht, :], accbf[:, ht * P:(ht + 1) * P]
                    )
                # tail: out = r_own ⊙ (accT.T @ W2T + v ⊗ b2)
                pt = ptail_pool.tile([P, OF], F32)
                for ht in range(HT):
                    nc.tensor.matmul(
                        pt, accT[:, ht, :], W2T[:, ht, :],
                        start=(ht == 0), stop=(ht == HT - 1),
                    )
                vb = vb_pool.tile([P, OF], BF16)
                nc.vector.tensor_scalar_mul(vb, b2_bcast, vcol)
                nc.vector.tensor_tensor(pt, pt, vb, mybir.AluOpType.add)
                o_sb = out_pool.tile([P, OF], F32)
                nc.vector.tensor_scalar_mul(o_sb, pt, r_own[:, it:it + 1])
                nc.sync.dma_start(out.ap()[it * P:(it + 1) * P, :], o_sb)

    nc.compile()
    return nc


_NC_CACHE = {}


def _get_nc(key=8192):
    if key not in _NC_CACHE:
        _NC_CACHE[key] = build_nc(n_nodes=key)
    return _NC_CACHE[key]


def kernel(x, adj, W1, b1, W2, b2):
    """Full-input entry point: shard internally across 8 cores, return full output."""
    n = adj.shape[0]
    nc = _get_nc(n)
    sh = n // NCORES
    x = np.ascontiguousarray(np.asarray(x, dtype=np.float32))
    adj = np.ascontiguousarray(np.asarray(adj, dtype=np.float32))
    W1 = np.ascontiguousarray(np.asarray(W1, dtype=np.float32))
    b1 = np.ascontiguousarray(np.asarray(b1, dtype=np.float32))
    W2 = np.ascontiguousarray(np.asarray(W2, dtype=np.float32))
    b2 = np.ascontiguousarray(np.asarray(b2, dtype=np.float32))
    in_maps = [
        {
            "adj_shard": adj[c * sh:(c + 1) * sh],
            "x_shard": x[c * sh:(c + 1) * sh],
            "W1": W1,
            "b1": b1,
            "W2": W2,
            "b2": b2,
        }
        for c in range(NCORES)
    ]
    res = run_bass_kernel_spmd(nc, in_maps, list(range(NCORES)), trace=False)
    return np.concatenate(
        [res.results[c]["out_shard"] for c in range(NCORES)], axis=0
    )


# revision 16
# speedup vs baseline: 2.4289x; 2.4289x over previous
"""GCN layer kernel for 8 Trainium2 NeuronCores.

Math (reference):
    h  = (x @ W1.T + b1) @ W2.T + b2
    A  = adj + I
    r  = rowsum(A) ** -0.5
    out = (r[:,None] * A * r[None,:]) @ h
        = r ⊙ (adj @ (r ⊙ h)) + r ⊙ (r ⊙ h)

Associativity: with h1 = x @ W1.T + b1 and g1 = [r ⊙ h1 | r] (257 cols),
    acc  = adj @ g1 + g1_own            # [rows, 257]  (spmm, K=8192)
    out  = r ⊙ (acc[:, :256] @ W2.T + acc[:, 256] ⊗ b2)

Distribution: row-shard adj and x across 8 cores (1024 rows each).
Per core:
  - stream the 32MB fp32 adj shard once; one ACT op per chunk does the
    bf16 cast + rowsum (accum_out); XBAR DMA-transposes (SBUF->SBUF, on
    the ACT HWDGE ring) land bf16 adjT [128, 64, 1024] resident in SBUF.
  - mm1 on the local x shard -> h1 (bf16, hid on partitions); AllGather
    h1 (1MB, overlaps the adj streaming).
  - AllGather rowsums (4KB); r = sqrt(1/(rowsum+1)).
  - g1 built by DMA-transposing gathered h1 + r scale.
  - spmm: per local i-tile accumulate 64 bf16 matmuls [128x128]@[128x257]
    into one PSUM bank; small @W2T tail + bias + r_i scale; DMA out.
"""

import sys

if "/opt/trn_rl_repo" not in sys.path:
    sys.path.insert(0, "/opt/trn_rl_repo")

import numpy as np

import concourse.bass as bass
import concourse.mybir as mybir
import concourse.tile as tile
from concourse import bacc
from concourse.bass_utils import run_bass_kernel_spmd
from concourse.masks import make_identity

F32 = mybir.dt.float32
BF16 = mybir.dt.bfloat16
P = 128
NCORES = 8


def build_nc(n_nodes=8192, in_f=512, hid=256, out_f=512, cw=2048, debug_dump=False):
    """Build and compile the per-core SPMD kernel."""
    cw = min(cw, n_nodes)
    SH = n_nodes // NCORES      # shard rows per core
    KB = SH // P                # row blocks in shard (= local i tiles)
    JT = n_nodes // P           # global j tiles
    QN = n_nodes // cw          # adj chunks per row block
    BPC = cw // P               # 128-blocks per chunk
    FT = in_f // P              # input feature tiles
    HT = hid // P               # hidden tiles
    OF = out_f
    GW = hid + 1                # g1 logical width: [r*h1 | r]
    GWP = hid + 16              # padded block stride (32B-aligned for tDMA dests)
    IG = min(4, KB)             # i-stripes per mm1 group
    NG = IG * P                 # mm1 rhs free size

    nc = bacc.Bacc(
        "TRN2",
        target_bir_lowering=False,
        debug=False,
        num_devices=NCORES,
        dynamic_dma_scratch_size=4096,
    )
    adj_s = nc.dram_tensor("adj_shard", [SH, n_nodes], F32, kind="ExternalInput")
    x_s = nc.dram_tensor("x_shard", [SH, in_f], F32, kind="ExternalInput")
    W1 = nc.dram_tensor("W1", [hid, in_f], F32, kind="ExternalInput")
    b1 = nc.dram_tensor("b1", [hid], F32, kind="ExternalInput")
    W2 = nc.dram_tensor("W2", [out_f, hid], F32, kind="ExternalInput")
    b2 = nc.dram_tensor("b2", [out_f], F32, kind="ExternalInput")
    out = nc.dram_tensor("out_shard", [SH, OF], F32, kind="ExternalOutput")

    with tile.TileContext(nc) as tc:
        with (
            tc.tile_pool(name="const", bufs=1) as cpool,
            tc.tile_pool(name="big", bufs=1) as bigpool,
            tc.tile_pool(name="adj_in", bufs=2) as adj_in_pool,
            tc.tile_pool(name="chbf", bufs=3) as chbf_pool,
            tc.tile_pool(name="xt", bufs=1) as xt_pool,
            tc.tile_pool(name="acc_bf", bufs=2) as accbf_pool,
            tc.tile_pool(name="vb", bufs=1) as vb_pool,
            tc.tile_pool(name="out_sb", bufs=2) as out_pool,
            tc.tile_pool(name="pmm", bufs=2, space="PSUM") as pmm_pool,
            tc.tile_pool(name="pacc", bufs=2, space="PSUM") as pacc_pool,
            tc.tile_pool(name="ptail", bufs=2, space="PSUM") as ptail_pool,
            tc.tile_pool(name="dram", bufs=1, space="DRAM") as dram,
        ):
            # ---------- constants ----------
            ones_bf = cpool.tile([1, max(OF, P)], BF16)
            nc.vector.memset(ones_bf, 1.0)
            # SWDGE DMA casts f32 -> bf16 in flight
            b1_bf = cpool.tile([1, hid], BF16)
            nc.gpsimd.dma_start(b1_bf, b1.ap()[None, :])
            b2_bf = cpool.tile([1, out_f], BF16)
            nc.gpsimd.dma_start(b2_bf, b2.ap()[None, :])
            # b2 broadcast to all partitions (for the v*b2 outer product)
            pb = pmm_pool.tile([P, OF], F32, tag="pmm")
            nc.tensor.matmul(pb, ones_bf[:1, :P], b2_bf[:1, :], start=True, stop=True)
            b2_bcast = cpool.tile([P, OF], BF16)
            nc.scalar.copy(b2_bcast, pb)

            # ---------- weights: W1T [P, FT, hid], W2T [P, HT, OF] via tDMA ----------
            W1T = cpool.tile([P, FT, hid], BF16)
            W2T = cpool.tile([P, HT, OF], BF16)
            for s in range(HT):  # W1 row stripes (hid)
                w_in = adj_in_pool.tile([P, cw], F32, tag="adj_in")
                nc.sync.dma_start(w_in[:, :in_f], W1.ap()[s * P:(s + 1) * P, :])
                w_bf = chbf_pool.tile([P, cw], BF16, tag="chbf")
                nc.scalar.copy(w_bf[:, :in_f], w_in[:, :in_f])
                nc.scalar.dma_start_transpose(
                    W1T[:, :, s * P:(s + 1) * P], w_bf[:, :in_f]
                )
            for s in range(out_f // P):  # W2 row stripes (out_f)
                w_in = adj_in_pool.tile([P, cw], F32, tag="adj_in")
                nc.sync.dma_start(w_in[:, :hid], W2.ap()[s * P:(s + 1) * P, :])
                w_bf = chbf_pool.tile([P, cw], BF16, tag="chbf")
                nc.scalar.copy(w_bf[:, :hid], w_in[:, :hid])
                nc.scalar.dma_start_transpose(
                    W2T[:, :, s * P:(s + 1) * P], w_bf[:, :hid]
                )

            # ---------- mm1: h1_c [P, HT, SH] bf16 (hid on partitions) ----------
            h1_c = bigpool.tile([P, HT, SH], BF16)
            for grp in range(SH // NG):
                xt = xt_pool.tile([P, FT, NG], BF16)
                for s in range(IG):
                    x_in = adj_in_pool.tile([P, cw], F32, tag="adj_in")
                    row0 = (grp * IG + s) * P
                    nc.sync.dma_start(x_in[:, :in_f], x_s.ap()[row0:row0 + P, :])
                    x_bf = chbf_pool.tile([P, cw], BF16, tag="chbf")
                    nc.scalar.copy(x_bf[:, :in_f], x_in[:, :in_f])
                    nc.scalar.dma_start_transpose(
                        xt[:, :, s * P:(s + 1) * P], x_bf[:, :in_f]
                    )
                for ht in range(HT):
                    pm = pmm_pool.tile([P, OF], F32, tag="pmm")
                    pm1 = pm[:, :NG]
                    nc.tensor.matmul(
                        pm1, b1_bf[:1, ht * P:(ht + 1) * P], ones_bf[:1, :NG],
                        start=True, stop=False,
                    )
                    for ft in range(FT):
                        nc.tensor.matmul(
                            pm1,
                            W1T[:, ft, ht * P:(ht + 1) * P],
                            xt[:, ft, :],
                            start=False, stop=(ft == FT - 1),
                        )
                    nc.scalar.copy(h1_c[:, ht, grp * NG:(grp + 1) * NG], pm1)

            # ---------- AllGather h1 ----------
            h1d = dram.tile([HT, P, SH], BF16)
            for ht in range(HT):
                nc.sync.dma_start(h1d[ht], h1_c[:, ht, :])
            h1g = dram.tile([NCORES * HT, P, SH], BF16, addr_space="Shared")
            nc.gpsimd.collective_compute(
                "AllGather",
                mybir.AluOpType.bypass,
                replica_groups=[list(range(NCORES))],
                ins=[h1d.opt()],
                outs=[h1g.opt()],
            )

            # ---------- stream adj shard: fused cast+rowsum, tDMA into adjT ----------
            adjT = bigpool.tile([P, JT, SH], BF16)
            rowsum_c = cpool.tile([P, KB], F32)
            for k in range(KB):
                partials = cpool.tile([P, QN], F32, tag="partials")
                for q in range(QN):
                    ch = adj_in_pool.tile([P, cw], F32, tag="adj_in")
                    nc.sync.dma_start(
                        ch, adj_s.ap()[k * P:(k + 1) * P, q * cw:(q + 1) * cw]
                    )
                    chbf = chbf_pool.tile([P, cw], BF16, tag="chbf")
                    nc.scalar.activation(
                        chbf, ch, mybir.ActivationFunctionType.Identity,
                        bias=0.0, scale=1.0, accum_out=partials[:, q:q + 1],
                    )
                    nc.scalar.dma_start_transpose(
                        adjT[:, q * BPC:(q + 1) * BPC, k * P:(k + 1) * P], chbf
                    )
                nc.vector.reduce_sum(
                    rowsum_c[:, k:k + 1], partials, axis=mybir.AxisListType.X
                )

            # ---------- AllGather rowsums; r vectors ----------
            rsd = dram.tile([P, KB], F32)
            nc.sync.dma_start(rsd, rowsum_c)
            rsg = dram.tile([NCORES * P, KB], F32, addr_space="Shared")
            nc.gpsimd.collective_compute(
                "AllGather",
                mybir.AluOpType.bypass,
                replica_groups=[list(range(NCORES))],
                ins=[rsd.opt()],
                outs=[rsg.opt()],
            )
            # r_sb [P, NCORES, KB]: r for global tile jt = (c, k) at [:, c, k]
            rs_t = cpool.tile([P, NCORES, KB], F32)
            nc.sync.dma_start(rs_t, rsg.rearrange("(c p) k -> p c k", p=P))
            r_sb = cpool.tile([P, NCORES, KB], F32)
            nc.vector.tensor_scalar_add(rs_t, rs_t, 1.0)
            nc.vector.reciprocal(rs_t, rs_t)
            nc.scalar.sqrt(r_sb, rs_t)
            # local r for own rows
            r_own = cpool.tile([P, KB], F32)
            ro_t = cpool.tile([P, KB], F32)
            nc.vector.tensor_scalar_add(ro_t, rowsum_c, 1.0)
            nc.vector.reciprocal(ro_t, ro_t)
            nc.scalar.sqrt(r_own, ro_t)

            # ---------- g1 [P, JT, GW] = [r ⊙ h1ᵀ | r] for all rows ----------
            g1 = bigpool.tile([P, JT, GWP], BF16)
            for c in range(NCORES):
                for ht in range(HT):
                    # dest[p, k, q] = h1g[c*HT+ht, q, k*P+p]
                    nc.scalar.dma_start_transpose(
                        g1[:, c * KB:(c + 1) * KB, ht * P:(ht + 1) * P],
                        h1g[c * HT + ht, :, :],
                    )
            for jt in range(JT):
                c, k = jt // KB, jt % KB
                rc = r_sb[:, c, k:k + 1]
                nc.vector.tensor_scalar_mul(g1[:, jt, :hid], g1[:, jt, :hid], rc)
                nc.vector.tensor_copy(g1[:, jt, hid:GW], rc)
            # own-row g1 from local h1 (identity term)
            g1own = bigpool.tile([P, KB, GWP], BF16)
            for ht in range(HT):
                nc.scalar.dma_start_transpose(
                    g1own[:, :, ht * P:(ht + 1) * P], h1_c[:, ht, :]
                )
            for k in range(KB):
                rc = r_own[:, k:k + 1]
                nc.vector.tensor_scalar_mul(g1own[:, k, :hid], g1own[:, k, :hid], rc)
                nc.vector.tensor_copy(g1own[:, k, hid:GW], rc)

            # ---------- debug dumps ----------
            if debug_dump:
                d_adjT = nc.dram_tensor(
                    "d_adjT", [P, JT, SH], BF16, kind="ExternalOutput"
                )
                nc.sync.dma_start(d_adjT.ap(), adjT)
                d_g1 = nc.dram_tensor("d_g1", [P, JT, GWP], BF16, kind="ExternalOutput")
                nc.sync.dma_start(d_g1.ap(), g1)
                d_h1 = nc.dram_tensor("d_h1", [P, HT, SH], BF16, kind="ExternalOutput")
                nc.sync.dma_start(d_h1.ap(), h1_c)
                d_rs = nc.dram_tensor("d_rs", [P, KB], F32, kind="ExternalOutput")
                nc.sync.dma_start(d_rs.ap(), rowsum_c)

            # ---------- spmm + tail ----------
            for it in range(KB):
                acc = pacc_pool.tile([P, GW], F32)
                for jt in range(JT):
                    nc.tensor.matmul(
                        acc,
                        adjT[:, jt, it * P:(it + 1) * P],
                        g1[:, jt, :GW],
                        start=(jt == 0),
                        stop=(jt == JT - 1),
                    )
                # acc += g1_own (identity term), cast to bf16 for the tail
                nc.vector.tensor_tensor(acc, acc, g1own[:, it, :GW], mybir.AluOpType.add)
                accbf = accbf_pool.tile([P, hid], BF16)
                nc.scalar.copy(accbf, acc[:, :hid])
                vcol = accbf_pool.tile([P, 1], F32, tag="vcol")
                nc.vector.tensor_copy(vcol, acc[:, hid:GW])
                # accT [P(h1), HT, P(i)] via SBUF->SBUF tDMA
                accT = accbf_pool.tile([P, HT, P], BF16, tag="accT")
                nc.scalar.dma_start_transpose(accT, accbf)
                # tail: out = r_own ⊙ (accT.T @ W2T + v ⊗ b2)
                pt = ptail_pool.tile([P, OF], F32)
                for ht in range(HT):
                    nc.tensor.matmul(
                        pt, accT[:, ht, :], W2T[:, ht, :],
                        start=(ht == 0), stop=(ht == HT - 1),
                    )
                vb = vb_pool.tile([P, OF], BF16)
                nc.vector.tensor_scalar_mul(vb, b2_bcast, vcol)
                nc.vector.tensor_tensor(pt, pt, vb, mybir.AluOpType.add)
                o_sb = out_pool.tile([P, OF], F32)
                nc.vector.tensor_scalar_mul(o_sb, pt, r_own[:, it:it + 1])
                nc.sync.dma_start(out.ap()[it * P:(it + 1) * P, :], o_sb)

    nc.compile()
    return nc


_NC_CACHE = {}


def _get_nc(key=8192):
    if key not in _NC_CACHE:
        _NC_CACHE[key] = build_nc(n_nodes=key)
    return _NC_CACHE[key]


def kernel(x, adj, W1, b1, W2, b2):
    """Full-input entry point: shard internally across 8 cores, return full output."""
    n = adj.shape[0]
    nc = _get_nc(n)
    sh = n // NCORES
    x = np.ascontiguousarray(np.asarray(x, dtype=np.float32))
    adj = np.ascontiguousarray(np.asarray(adj, dtype=np.float32))
    W1 = np.ascontiguousarray(np.asarray(W1, dtype=np.float32))
    b1 = np.ascontiguousarray(np.asarray(b1, dtype=np.float32))
    W2 = np.ascontiguousarray(np.asarray(W2, dtype=np.float32))
    b2 = np.ascontiguousarray(np.asarray(b2, dtype=np.float32))
    in_maps = [
        {
            "adj_shard": adj[c * sh:(c + 1) * sh],
            "x_shard": x[c * sh:(c + 1) * sh],
            "W1": W1,
            "b1": b1,
            "W2": W2,
            "b2": b2,
        }
        for c in range(NCORES)
    ]
    res = run_bass_kernel_spmd(nc, in_maps, list(range(NCORES)), trace=False)
    return np.concatenate(
        [res.results[c]["out_shard"] for c in range(NCORES)], axis=0
    )


# revision 19
# speedup vs baseline: 2.6503x; 1.0912x over previous
"""GCN layer kernel for 8 Trainium2 NeuronCores.

Math (reference):
    h  = (x @ W1.T + b1) @ W2.T + b2
    A  = adj + I
    r  = rowsum(A) ** -0.5
    out = (r[:,None] * A * r[None,:]) @ h
        = r ⊙ (adj @ (r ⊙ h)) + r ⊙ (r ⊙ h)

Associativity: with h1 = x @ W1.T + b1 and g1 = [r ⊙ h1 | r] (257 cols),
    acc  = adj @ g1 + g1_own            # [rows, 257]  (spmm, K=8192)
    out  = r ⊙ (acc[:, :256] @ W2.T + acc[:, 256] ⊗ b2)

Distribution: row-shard adj and x across 8 cores (1024 rows each).
Per core:
  - stream the 32MB fp32 adj shard once; one ACT op per chunk does the
    bf16 cast + rowsum (accum_out); XBAR DMA-transposes (SBUF->SBUF, on
    the ACT HWDGE ring) land bf16 adjT [128, 64, 1024] resident in SBUF.
  - mm1 on the local x shard -> h1 (bf16, hid on partitions); AllGather
    h1 (1MB, overlaps the adj streaming).
  - AllGather rowsums (4KB); r = sqrt(1/(rowsum+1)).
  - g1 built by DMA-transposing gathered h1 + r scale.
  - spmm: per local i-tile accumulate 64 bf16 matmuls [128x128]@[128x257]
    into one PSUM bank; small @W2T tail + bias + r_i scale; DMA out.
"""

import sys

if "/opt/trn_rl_repo" not in sys.path:
    sys.path.insert(0, "/opt/trn_rl_repo")

import numpy as np

import concourse.bass as bass
import concourse.mybir as mybir
import concourse.tile as tile
from concourse import bacc
from concourse.bass_utils import run_bass_kernel_spmd
from concourse.masks import make_identity

F32 = mybir.dt.float32
BF16 = mybir.dt.bfloat16
P = 128
NCORES = 8


def build_nc(n_nodes=8192, in_f=512, hid=256, out_f=512, cw=2048, debug_dump=False):
    """Build and compile the per-core SPMD kernel."""
    cw = min(cw, n_nodes)
    SH = n_nodes // NCORES      # shard rows per core
    KB = SH // P                # row blocks in shard (= local i tiles)
    JT = n_nodes // P           # global j tiles
    QN = n_nodes // cw          # adj chunks per row block
    HALF = min(2 * cw, n_nodes)  # bf16 staging width (fewer, bigger tDMAs)
    NH = n_nodes // HALF        # staging buffers per row block
    QPH = HALF // cw            # read-chunks per staging buffer
    BPH = HALF // P             # 128-blocks per staging buffer
    FT = in_f // P              # input feature tiles
    HT = hid // P               # hidden tiles
    OF = out_f
    GW = hid + 1                # g1 logical width: [r*h1 | r]
    GWP = hid + 16              # padded block stride (32B-aligned for tDMA dests)
    IG = min(2, KB)             # i-stripes per mm1 group
    NG = IG * P                 # mm1 rhs free size

    nc = bacc.Bacc(
        "TRN2",
        target_bir_lowering=False,
        debug=False,
        num_devices=NCORES,
        dynamic_dma_scratch_size=4096,
    )
    adj_s = nc.dram_tensor("adj_shard", [SH, n_nodes], F32, kind="ExternalInput")
    x_s = nc.dram_tensor("x_shard", [SH, in_f], F32, kind="ExternalInput")
    W1 = nc.dram_tensor("W1", [hid, in_f], F32, kind="ExternalInput")
    b1 = nc.dram_tensor("b1", [hid], F32, kind="ExternalInput")
    W2 = nc.dram_tensor("W2", [out_f, hid], F32, kind="ExternalInput")
    b2 = nc.dram_tensor("b2", [out_f], F32, kind="ExternalInput")
    out = nc.dram_tensor("out_shard", [SH, OF], F32, kind="ExternalOutput")

    with tile.TileContext(nc) as tc:
        with (
            tc.tile_pool(name="const", bufs=1) as cpool,
            tc.tile_pool(name="big", bufs=1) as bigpool,
            tc.tile_pool(name="adj_in", bufs=2) as adj_in_pool,
            tc.tile_pool(name="chbf", bufs=2) as chbf_pool,
            tc.tile_pool(name="xt", bufs=1) as xt_pool,
            tc.tile_pool(name="acc_bf", bufs=1) as accbf_pool,
            tc.tile_pool(name="vb", bufs=1) as vb_pool,
            tc.tile_pool(name="out_sb", bufs=1) as out_pool,
            tc.tile_pool(name="pmm", bufs=2, space="PSUM") as pmm_pool,
            tc.tile_pool(name="pacc", bufs=2, space="PSUM") as pacc_pool,
            tc.tile_pool(name="ptail", bufs=2, space="PSUM") as ptail_pool,
            tc.tile_pool(name="dram", bufs=1, space="DRAM") as dram,
        ):
            # ---------- constants ----------
            ones_bf = cpool.tile([1, max(OF, P)], BF16)
            nc.vector.memset(ones_bf, 1.0)
            # SWDGE DMA casts f32 -> bf16 in flight
            b1_bf = cpool.tile([1, hid], BF16)
            nc.gpsimd.dma_start(b1_bf, b1.ap()[None, :])
            b2_bf = cpool.tile([1, out_f], BF16)
            nc.gpsimd.dma_start(b2_bf, b2.ap()[None, :])
            # b2 broadcast to all partitions (for the v*b2 outer product)
            pb = pmm_pool.tile([P, OF], F32, tag="pmm")
            nc.tensor.matmul(pb, ones_bf[:1, :P], b2_bf[:1, :], start=True, stop=True)
            b2_bcast = cpool.tile([P, OF], BF16)
            nc.scalar.copy(b2_bcast, pb)

            # ---------- weights: W1T [P, FT, hid], W2T [P, HT, OF] via tDMA ----------
            W1T = cpool.tile([P, FT, hid], BF16)
            W2T = cpool.tile([P, HT, OF], BF16)
            for s in range(HT):  # W1 row stripes (hid)
                w_in = adj_in_pool.tile([P, cw], F32, tag="adj_in")
                nc.sync.dma_start(w_in[:, :in_f], W1.ap()[s * P:(s + 1) * P, :])
                w_bf = chbf_pool.tile([P, cw], BF16, tag="chbf")
                nc.scalar.copy(w_bf[:, :in_f], w_in[:, :in_f])
                nc.scalar.dma_start_transpose(
                    W1T[:, :, s * P:(s + 1) * P], w_bf[:, :in_f]
                )
            for s in range(out_f // P):  # W2 row stripes (out_f)
                w_in = adj_in_pool.tile([P, cw], F32, tag="adj_in")
                nc.sync.dma_start(w_in[:, :hid], W2.ap()[s * P:(s + 1) * P, :])
                w_bf = chbf_pool.tile([P, cw], BF16, tag="chbf")
                nc.scalar.copy(w_bf[:, :hid], w_in[:, :hid])
                nc.scalar.dma_start_transpose(
                    W2T[:, :, s * P:(s + 1) * P], w_bf[:, :hid]
                )

            # ---------- mm1: h1_c [P, HT, SH] bf16 (hid on partitions) ----------
            h1_c = bigpool.tile([P, HT, SH], BF16)
            for grp in range(SH // NG):
                xt = xt_pool.tile([P, FT, NG], BF16)
                for s in range(IG):
                    x_in = adj_in_pool.tile([P, cw], F32, tag="adj_in")
                    row0 = (grp * IG + s) * P
                    nc.sync.dma_start(x_in[:, :in_f], x_s.ap()[row0:row0 + P, :])
                    x_bf = chbf_pool.tile([P, cw], BF16, tag="chbf")
                    nc.scalar.copy(x_bf[:, :in_f], x_in[:, :in_f])
                    nc.scalar.dma_start_transpose(
                        xt[:, :, s * P:(s + 1) * P], x_bf[:, :in_f]
                    )
                for ht in range(HT):
                    pm = pmm_pool.tile([P, OF], F32, tag="pmm")
                    pm1 = pm[:, :NG]
                    nc.tensor.matmul(
                        pm1, b1_bf[:1, ht * P:(ht + 1) * P], ones_bf[:1, :NG],
                        start=True, stop=False,
                    )
                    for ft in range(FT):
                        nc.tensor.matmul(
                            pm1,
                            W1T[:, ft, ht * P:(ht + 1) * P],
                            xt[:, ft, :],
                            start=False, stop=(ft == FT - 1),
                        )
                    nc.scalar.copy(h1_c[:, ht, grp * NG:(grp + 1) * NG], pm1)

            # ---------- AllGather h1 ----------
            h1d = dram.tile([HT, P, SH], BF16)
            for ht in range(HT):
                nc.sync.dma_start(h1d[ht], h1_c[:, ht, :])
            h1g = dram.tile([NCORES * HT, P, SH], BF16, addr_space="Shared")
            nc.gpsimd.collective_compute(
                "AllGather",
                mybir.AluOpType.bypass,
                replica_groups=[list(range(NCORES))],
                ins=[h1d.opt()],
                outs=[h1g.opt()],
            )

            # ---------- stream adj shard: fused cast+rowsum, tDMA into adjT ----------
            adjT = bigpool.tile([P, JT, SH], BF16)
            rowsum_c = cpool.tile([P, KB], F32)
            for k in range(KB):
                partials = cpool.tile([P, QN], F32, tag="partials")
                for h in range(NH):
                    chbf = chbf_pool.tile([P, HALF], BF16, tag="chbf")
                    for qq in range(QPH):
                        q = h * QPH + qq
                        ch = adj_in_pool.tile([P, cw], F32, tag="adj_in")
                        nc.sync.dma_start(
                            ch, adj_s.ap()[k * P:(k + 1) * P, q * cw:(q + 1) * cw]
                        )
                        dst = chbf[:, qq * cw:(qq + 1) * cw]
                        if (k * QN + q) % 2 == 0:
                            # ACT: fused cast + rowsum
                            nc.scalar.activation(
                                dst, ch, mybir.ActivationFunctionType.Identity,
                                bias=0.0, scale=1.0,
                                accum_out=partials[:, q:q + 1],
                            )
                        else:
                            # DVE: cast, then reduce the bf16 copy
                            nc.vector.tensor_copy(dst, ch)
                            nc.vector.reduce_sum(
                                partials[:, q:q + 1], dst,
                                axis=mybir.AxisListType.X,
                            )
                    nc.scalar.dma_start_transpose(
                        adjT[:, h * BPH:(h + 1) * BPH, k * P:(k + 1) * P], chbf
                    )
                nc.vector.reduce_sum(
                    rowsum_c[:, k:k + 1], partials, axis=mybir.AxisListType.X
                )

            # ---------- AllGather rowsums; r vectors ----------
            rsd = dram.tile([P, KB], F32)
            nc.sync.dma_start(rsd, rowsum_c)
            rsg = dram.tile([NCORES * P, KB], F32, addr_space="Shared")
            nc.gpsimd.collective_compute(
                "AllGather",
                mybir.AluOpType.bypass,
                replica_groups=[list(range(NCORES))],
                ins=[rsd.opt()],
                outs=[rsg.opt()],
            )
            # r_sb [P, NCORES, KB]: r for global tile jt = (c, k) at [:, c, k]
            rs_t = cpool.tile([P, NCORES, KB], F32)
            nc.sync.dma_start(rs_t, rsg.rearrange("(c p) k -> p c k", p=P))
            r_sb = cpool.tile([P, NCORES, KB], F32)
            nc.vector.tensor_scalar_add(rs_t, rs_t, 1.0)
            nc.vector.reciprocal(rs_t, rs_t)
            nc.scalar.sqrt(r_sb, rs_t)
            # local r for own rows
            r_own = cpool.tile([P, KB], F32)
            ro_t = cpool.tile([P, KB], F32)
            nc.vector.tensor_scalar_add(ro_t, rowsum_c, 1.0)
            nc.vector.reciprocal(ro_t, ro_t)
            nc.scalar.sqrt(r_own, ro_t)

            # ---------- g1 [P, JT, GW] = [r ⊙ h1ᵀ | r] for all rows ----------
            g1 = bigpool.tile([P, JT, GWP], BF16)
            for c in range(NCORES):
                for ht in range(HT):
                    # dest[p, k, q] = h1g[c*HT+ht, q, k*P+p]
                    nc.scalar.dma_start_transpose(
                        g1[:, c * KB:(c + 1) * KB, ht * P:(ht + 1) * P],
                        h1g[c * HT + ht, :, :],
                    )
            for jt in range(JT):
                c, k = jt // KB, jt % KB
                rc = r_sb[:, c, k:k + 1]
                nc.vector.tensor_scalar_mul(g1[:, jt, :hid], g1[:, jt, :hid], rc)
                nc.vector.tensor_copy(g1[:, jt, hid:GW], rc)
            # own-row g1 from local h1 (identity term)
            g1own = bigpool.tile([P, KB, GWP], BF16)
            for ht in range(HT):
                nc.scalar.dma_start_transpose(
                    g1own[:, :, ht * P:(ht + 1) * P], h1_c[:, ht, :]
                )
            for k in range(KB):
                rc = r_own[:, k:k + 1]
                nc.vector.tensor_scalar_mul(g1own[:, k, :hid], g1own[:, k, :hid], rc)
                nc.vector.tensor_copy(g1own[:, k, hid:GW], rc)

            # ---------- debug dumps ----------
            if debug_dump:
                d_adjT = nc.dram_tensor(
                    "d_adjT", [P, JT, SH], BF16, kind="ExternalOutput"
                )
                nc.sync.dma_start(d_adjT.ap(), adjT)
                d_g1 = nc.dram_tensor("d_g1", [P, JT, GWP], BF16, kind="ExternalOutput")
                nc.sync.dma_start(d_g1.ap(), g1)
                d_h1 = nc.dram_tensor("d_h1", [P, HT, SH], BF16, kind="ExternalOutput")
                nc.sync.dma_start(d_h1.ap(), h1_c)
                d_rs = nc.dram_tensor("d_rs", [P, KB], F32, kind="ExternalOutput")
                nc.sync.dma_start(d_rs.ap(), rowsum_c)

            # ---------- spmm + tail ----------
            for it in range(KB):
                acc = pacc_pool.tile([P, GW], F32)
                for jt in range(JT):
                    nc.tensor.matmul(
                        acc,
                        adjT[:, jt, it * P:(it + 1) * P],
                        g1[:, jt, :GW],
                        start=(jt == 0),
                        stop=(jt == JT - 1),
                    )
                # acc += g1_own (identity term), cast to bf16 for the tail
                nc.vector.tensor_tensor(acc, acc, g1own[:, it, :GW], mybir.AluOpType.add)
                accbf = accbf_pool.tile([P, hid], BF16)
                nc.scalar.copy(accbf, acc[:, :hid])
                vcol = accbf_pool.tile([P, 1], F32, tag="vcol")
                nc.vector.tensor_copy(vcol, acc[:, hid:GW])
                # accT [P(h1), HT, P(i)] via SBUF->SBUF tDMA
                accT = accbf_pool.tile([P, HT, P], BF16, tag="accT")
                nc.scalar.dma_start_transpose(accT, accbf)
                # tail: out = r_own ⊙ (accT.T @ W2T + v ⊗ b2)
                pt = ptail_pool.tile([P, OF], F32)
                for ht in range(HT):
                    nc.tensor.matmul(
                        pt, accT[:, ht, :], W2T[:, ht, :],
                        start=(ht == 0), stop=(ht == HT - 1),
                    )
                vb = vb_pool.tile([P, OF], BF16)
                nc.vector.tensor_scalar_mul(vb, b2_bcast, vcol)
                nc.vector.tensor_tensor(pt, pt, vb, mybir.AluOpType.add)
                o_sb = out_pool.tile([P, OF], F32)
                nc.vector.tensor_scalar_mul(o_sb, pt, r_own[:, it:it + 1])
                nc.sync.dma_start(out.ap()[it * P:(it + 1) * P, :], o_sb)

    nc.compile()
    return nc


_NC_CACHE = {}


def _get_nc(key=8192):
    if key not in _NC_CACHE:
        _NC_CACHE[key] = build_nc(n_nodes=key)
    return _NC_CACHE[key]


def kernel(x, adj, W1, b1, W2, b2):
    """Full-input entry point: shard internally across 8 cores, return full output."""
    n = adj.shape[0]
    nc = _get_nc(n)
    sh = n // NCORES
    x = np.ascontiguousarray(np.asarray(x, dtype=np.float32))
    adj = np.ascontiguousarray(np.asarray(adj, dtype=np.float32))
    W1 = np.ascontiguousarray(np.asarray(W1, dtype=np.float32))
    b1 = np.ascontiguousarray(np.asarray(b1, dtype=np.float32))
    W2 = np.ascontiguousarray(np.asarray(W2, dtype=np.float32))
    b2 = np.ascontiguousarray(np.asarray(b2, dtype=np.float32))
    in_maps = [
        {
            "adj_shard": adj[c * sh:(c + 1) * sh],
            "x_shard": x[c * sh:(c + 1) * sh],
            "W1": W1,
            "b1": b1,
            "W2": W2,
            "b2": b2,
        }
        for c in range(NCORES)
    ]
    res = run_bass_kernel_spmd(nc, in_maps, list(range(NCORES)), trace=False)
    return np.concatenate(
        [res.results[c]["out_shard"] for c in range(NCORES)], axis=0
    )


# revision 20
# speedup vs baseline: 2.7227x; 1.0273x over previous
"""GCN layer kernel for 8 Trainium2 NeuronCores.

Math (reference):
    h  = (x @ W1.T + b1) @ W2.T + b2
    A  = adj + I
    r  = rowsum(A) ** -0.5
    out = (r[:,None] * A * r[None,:]) @ h
        = r ⊙ (adj @ (r ⊙ h)) + r ⊙ (r ⊙ h)

Associativity: with h1 = x @ W1.T + b1 and g1 = [r ⊙ h1 | r] (257 cols),
    acc  = adj @ g1 + g1_own            # [rows, 257]  (spmm, K=8192)
    out  = r ⊙ (acc[:, :256] @ W2.T + acc[:, 256] ⊗ b2)

Distribution: row-shard adj and x across 8 cores (1024 rows each).
Per core:
  - stream the 32MB fp32 adj shard once; one ACT op per chunk does the
    bf16 cast + rowsum (accum_out); XBAR DMA-transposes (SBUF->SBUF, on
    the ACT HWDGE ring) land bf16 adjT [128, 64, 1024] resident in SBUF.
  - mm1 on the local x shard -> h1 (bf16, hid on partitions); AllGather
    h1 (1MB, overlaps the adj streaming).
  - AllGather rowsums (4KB); r = sqrt(1/(rowsum+1)).
  - g1 built by DMA-transposing gathered h1 + r scale.
  - spmm: per local i-tile accumulate 64 bf16 matmuls [128x128]@[128x257]
    into one PSUM bank; small @W2T tail + bias + r_i scale; DMA out.
"""

import sys

if "/opt/trn_rl_repo" not in sys.path:
    sys.path.insert(0, "/opt/trn_rl_repo")

import numpy as np

import concourse.bass as bass
import concourse.mybir as mybir
import concourse.tile as tile
from concourse import bacc
from concourse.bass_utils import run_bass_kernel_spmd
from concourse.masks import make_identity

F32 = mybir.dt.float32
BF16 = mybir.dt.bfloat16
P = 128
NCORES = 8


def build_nc(n_nodes=8192, in_f=512, hid=256, out_f=512, cw=2048, debug_dump=False):
    """Build and compile the per-core SPMD kernel."""
    cw = min(cw, n_nodes)
    SH = n_nodes // NCORES      # shard rows per core
    KB = SH // P                # row blocks in shard (= local i tiles)
    JT = n_nodes // P           # global j tiles
    QN = n_nodes // cw          # adj chunks per row block
    HALF = min(2 * cw, n_nodes)  # bf16 staging width (fewer, bigger tDMAs)
    NH = n_nodes // HALF        # staging buffers per row block
    QPH = HALF // cw            # read-chunks per staging buffer
    BPH = HALF // P             # 128-blocks per staging buffer
    FT = in_f // P              # input feature tiles
    HT = hid // P               # hidden tiles
    OF = out_f
    GW = hid + 1                # g1 logical width: [r*h1 | r]
    GWP = hid + 16              # padded block stride (32B-aligned for tDMA dests)
    IG = min(2, KB)             # i-stripes per mm1 group
    NG = IG * P                 # mm1 rhs free size

    nc = bacc.Bacc(
        "TRN2",
        target_bir_lowering=False,
        debug=False,
        num_devices=NCORES,
        dynamic_dma_scratch_size=4096,
    )
    adj_s = nc.dram_tensor("adj_shard", [SH, n_nodes], F32, kind="ExternalInput")
    x_s = nc.dram_tensor("x_shard", [SH, in_f], F32, kind="ExternalInput")
    W1 = nc.dram_tensor("W1", [hid, in_f], F32, kind="ExternalInput")
    b1 = nc.dram_tensor("b1", [hid], F32, kind="ExternalInput")
    W2 = nc.dram_tensor("W2", [out_f, hid], F32, kind="ExternalInput")
    b2 = nc.dram_tensor("b2", [out_f], F32, kind="ExternalInput")
    out = nc.dram_tensor("out_shard", [SH, OF], F32, kind="ExternalOutput")

    with tile.TileContext(nc) as tc:
        with (
            tc.tile_pool(name="const", bufs=1) as cpool,
            tc.tile_pool(name="big", bufs=1) as bigpool,
            tc.tile_pool(name="chbf", bufs=3) as chbf_pool,
            tc.tile_pool(name="xt", bufs=1) as xt_pool,
            tc.tile_pool(name="acc_bf", bufs=1) as accbf_pool,
            tc.tile_pool(name="vb", bufs=1) as vb_pool,
            tc.tile_pool(name="out_sb", bufs=1) as out_pool,
            tc.tile_pool(name="pmm", bufs=2, space="PSUM") as pmm_pool,
            tc.tile_pool(name="pacc", bufs=2, space="PSUM") as pacc_pool,
            tc.tile_pool(name="ptail", bufs=2, space="PSUM") as ptail_pool,
            tc.tile_pool(name="dram", bufs=1, space="DRAM") as dram,
        ):
            # ---------- constants ----------
            ones_bf = cpool.tile([1, max(OF, P)], BF16)
            nc.vector.memset(ones_bf, 1.0)
            ones_col = cpool.tile([P, 1], BF16)
            nc.vector.memset(ones_col, 1.0)
            # SWDGE DMA casts f32 -> bf16 in flight
            b1_bf = cpool.tile([1, hid], BF16)
            nc.gpsimd.dma_start(b1_bf, b1.ap()[None, :])
            b2_bf = cpool.tile([1, out_f], BF16)
            nc.gpsimd.dma_start(b2_bf, b2.ap()[None, :])
            # b2 broadcast to all partitions (for the v*b2 outer product)
            pb = pmm_pool.tile([P, OF], F32, tag="pmm")
            nc.tensor.matmul(pb, ones_bf[:1, :P], b2_bf[:1, :], start=True, stop=True)
            b2_bcast = cpool.tile([P, OF], BF16)
            nc.scalar.copy(b2_bcast, pb)

            # ---------- weights: W1T [P, FT, hid], W2T [P, HT, OF] via tDMA ----------
            W1T = cpool.tile([P, FT, hid], BF16)
            W2T = cpool.tile([P, HT, OF], BF16)
            for s in range(HT):  # W1 row stripes (hid)
                w_bf = chbf_pool.tile([P, HALF], BF16, tag="chbf")
                nc.gpsimd.dma_start(w_bf[:, :in_f], W1.ap()[s * P:(s + 1) * P, :])
                nc.scalar.dma_start_transpose(
                    W1T[:, :, s * P:(s + 1) * P], w_bf[:, :in_f]
                )
            for s in range(out_f // P):  # W2 row stripes (out_f)
                w_bf = chbf_pool.tile([P, HALF], BF16, tag="chbf")
                nc.gpsimd.dma_start(w_bf[:, :hid], W2.ap()[s * P:(s + 1) * P, :])
                nc.scalar.dma_start_transpose(
                    W2T[:, :, s * P:(s + 1) * P], w_bf[:, :hid]
                )

            # ---------- mm1: h1_c [P, HT, SH] bf16 (hid on partitions) ----------
            h1_c = bigpool.tile([P, HT, SH], BF16)
            for grp in range(SH // NG):
                xt = xt_pool.tile([P, FT, NG], BF16)
                for s in range(IG):
                    row0 = (grp * IG + s) * P
                    x_bf = chbf_pool.tile([P, HALF], BF16, tag="chbf")
                    nc.gpsimd.dma_start(x_bf[:, :in_f], x_s.ap()[row0:row0 + P, :])
                    nc.scalar.dma_start_transpose(
                        xt[:, :, s * P:(s + 1) * P], x_bf[:, :in_f]
                    )
                for ht in range(HT):
                    pm = pmm_pool.tile([P, OF], F32, tag="pmm")
                    pm1 = pm[:, :NG]
                    nc.tensor.matmul(
                        pm1, b1_bf[:1, ht * P:(ht + 1) * P], ones_bf[:1, :NG],
                        start=True, stop=False,
                    )
                    for ft in range(FT):
                        nc.tensor.matmul(
                            pm1,
                            W1T[:, ft, ht * P:(ht + 1) * P],
                            xt[:, ft, :],
                            start=False, stop=(ft == FT - 1),
                        )
                    nc.scalar.copy(h1_c[:, ht, grp * NG:(grp + 1) * NG], pm1)

            # ---------- AllGather h1 ----------
            h1d = dram.tile([HT, P, SH], BF16)
            for ht in range(HT):
                nc.sync.dma_start(h1d[ht], h1_c[:, ht, :])
            h1g = dram.tile([NCORES * HT, P, SH], BF16, addr_space="Shared")
            nc.gpsimd.collective_compute(
                "AllGather",
                mybir.AluOpType.bypass,
                replica_groups=[list(range(NCORES))],
                ins=[h1d.opt()],
                outs=[h1g.opt()],
            )

            # ---------- stream adj shard: fused cast+rowsum, tDMA into adjT ----------
            adjT = bigpool.tile([P, JT, SH], BF16)
            rowsum_c = cpool.tile([P, KB], F32)
            for k in range(KB):
                for h in range(NH):
                    chbf = chbf_pool.tile([P, HALF], BF16, tag="chbf")
                    # SWDGE cast-DMA: fp32 HBM -> bf16 SBUF in one hop
                    nc.gpsimd.dma_start(
                        chbf, adj_s.ap()[k * P:(k + 1) * P, h * HALF:(h + 1) * HALF]
                    )
                    nc.scalar.dma_start_transpose(
                        adjT[:, h * BPH:(h + 1) * BPH, k * P:(k + 1) * P], chbf
                    )
                # rowsums on PE: psum[:, 0] += adjT[:, jt, kslice].T @ ones
                prs = pacc_pool.tile([P, 1], F32, tag="prs")
                for jt in range(JT):
                    nc.tensor.matmul(
                        prs,
                        adjT[:, jt, k * P:(k + 1) * P],
                        ones_col,
                        start=(jt == 0),
                        stop=(jt == JT - 1),
                    )
                nc.vector.tensor_copy(rowsum_c[:, k:k + 1], prs)

            # ---------- AllGather rowsums; r vectors ----------
            rsd = dram.tile([P, KB], F32)
            nc.sync.dma_start(rsd, rowsum_c)
            rsg = dram.tile([NCORES * P, KB], F32, addr_space="Shared")
            nc.gpsimd.collective_compute(
                "AllGather",
                mybir.AluOpType.bypass,
                replica_groups=[list(range(NCORES))],
                ins=[rsd.opt()],
                outs=[rsg.opt()],
            )
            # r_sb [P, NCORES, KB]: r for global tile jt = (c, k) at [:, c, k]
            rs_t = cpool.tile([P, NCORES, KB], F32)
            nc.sync.dma_start(rs_t, rsg.rearrange("(c p) k -> p c k", p=P))
            r_sb = cpool.tile([P, NCORES, KB], F32)
            nc.vector.tensor_scalar_add(rs_t, rs_t, 1.0)
            nc.vector.reciprocal(rs_t, rs_t)
            nc.scalar.sqrt(r_sb, rs_t)
            # local r for own rows
            r_own = cpool.tile([P, KB], F32)
            ro_t = cpool.tile([P, KB], F32)
            nc.vector.tensor_scalar_add(ro_t, rowsum_c, 1.0)
            nc.vector.reciprocal(ro_t, ro_t)
            nc.scalar.sqrt(r_own, ro_t)

            # ---------- g1 [P, JT, GW] = [r ⊙ h1ᵀ | r] for all rows ----------
            g1 = bigpool.tile([P, JT, GWP], BF16)
            for c in range(NCORES):
                for ht in range(HT):
                    # dest[p, k, q] = h1g[c*HT+ht, q, k*P+p]
                    nc.scalar.dma_start_transpose(
                        g1[:, c * KB:(c + 1) * KB, ht * P:(ht + 1) * P],
                        h1g[c * HT + ht, :, :],
                    )
            for jt in range(JT):
                c, k = jt // KB, jt % KB
                rc = r_sb[:, c, k:k + 1]
                nc.vector.tensor_scalar_mul(g1[:, jt, :hid], g1[:, jt, :hid], rc)
                nc.vector.tensor_copy(g1[:, jt, hid:GW], rc)
            # own-row g1 from local h1 (identity term)
            g1own = bigpool.tile([P, KB, GWP], BF16)
            for ht in range(HT):
                nc.scalar.dma_start_transpose(
                    g1own[:, :, ht * P:(ht + 1) * P], h1_c[:, ht, :]
                )
            for k in range(KB):
                rc = r_own[:, k:k + 1]
                nc.vector.tensor_scalar_mul(g1own[:, k, :hid], g1own[:, k, :hid], rc)
                nc.vector.tensor_copy(g1own[:, k, hid:GW], rc)

            # ---------- debug dumps ----------
            if debug_dump:
                d_adjT = nc.dram_tensor(
                    "d_adjT", [P, JT, SH], BF16, kind="ExternalOutput"
                )
                nc.sync.dma_start(d_adjT.ap(), adjT)
                d_g1 = nc.dram_tensor("d_g1", [P, JT, GWP], BF16, kind="ExternalOutput")
                nc.sync.dma_start(d_g1.ap(), g1)
                d_h1 = nc.dram_tensor("d_h1", [P, HT, SH], BF16, kind="ExternalOutput")
                nc.sync.dma_start(d_h1.ap(), h1_c)
                d_rs = nc.dram_tensor("d_rs", [P, KB], F32, kind="ExternalOutput")
                nc.sync.dma_start(d_rs.ap(), rowsum_c)

            # ---------- spmm + tail ----------
            for it in range(KB):
                acc = pacc_pool.tile([P, GW], F32)
                for jt in range(JT):
                    nc.tensor.matmul(
                        acc,
                        adjT[:, jt, it * P:(it + 1) * P],
                        g1[:, jt, :GW],
                        start=(jt == 0),
                        stop=(jt == JT - 1),
                    )
                # acc += g1_own (identity term), cast to bf16 for the tail
                nc.vector.tensor_tensor(acc, acc, g1own[:, it, :GW], mybir.AluOpType.add)
                accbf = accbf_pool.tile([P, hid], BF16)
                nc.scalar.copy(accbf, acc[:, :hid])
                vcol = accbf_pool.tile([P, 1], F32, tag="vcol")
                nc.vector.tensor_copy(vcol, acc[:, hid:GW])
                # accT [P(h1), HT, P(i)] via SBUF->SBUF tDMA
                accT = accbf_pool.tile([P, HT, P], BF16, tag="accT")
                nc.scalar.dma_start_transpose(accT, accbf)
                # tail: out = r_own ⊙ (accT.T @ W2T + v ⊗ b2)
                pt = ptail_pool.tile([P, OF], F32)
                for ht in range(HT):
                    nc.tensor.matmul(
                        pt, accT[:, ht, :], W2T[:, ht, :],
                        start=(ht == 0), stop=(ht == HT - 1),
                    )
                vb = vb_pool.tile([P, OF], BF16)
                nc.vector.tensor_scalar_mul(vb, b2_bcast, vcol)
                nc.vector.tensor_tensor(pt, pt, vb, mybir.AluOpType.add)
                o_sb = out_pool.tile([P, OF], F32)
                nc.vector.tensor_scalar_mul(o_sb, pt, r_own[:, it:it + 1])
                nc.sync.dma_start(out.ap()[it * P:(it + 1) * P, :], o_sb)

    nc.compile()
    return nc


_NC_CACHE = {}


def _get_nc(key=8192):
    if key not in _NC_CACHE:
        _NC_CACHE[key] = build_nc(n_nodes=key)
    return _NC_CACHE[key]


def kernel(x, adj, W1, b1, W2, b2):
    """Full-input entry point: shard internally across 8 cores, return full output."""
    n = adj.shape[0]
    nc = _get_nc(n)
    sh = n // NCORES
    x = np.ascontiguousarray(np.asarray(x, dtype=np.float32))
    adj = np.ascontiguousarray(np.asarray(adj, dtype=np.float32))
    W1 = np.ascontiguousarray(np.asarray(W1, dtype=np.float32))
    b1 = np.ascontiguousarray(np.asarray(b1, dtype=np.float32))
    W2 = np.ascontiguousarray(np.asarray(W2, dtype=np.float32))
    b2 = np.ascontiguousarray(np.asarray(b2, dtype=np.float32))
    in_maps = [
        {
            "adj_shard": adj[c * sh:(c + 1) * sh],
            "x_shard": x[c * sh:(c + 1) * sh],
            "W1": W1,
            "b1": b1,
            "W2": W2,
            "b2": b2,
        }
        for c in range(NCORES)
    ]
    res = run_bass_kernel_spmd(nc, in_maps, list(range(NCORES)), trace=False)
    return np.concatenate(
        [res.results[c]["out_shard"] for c in range(NCORES)], axis=0
    )


# revision 21
# speedup vs baseline: 2.7830x; 1.0221x over previous
"""GCN layer kernel for 8 Trainium2 NeuronCores.

Math (reference):
    h  = (x @ W1.T + b1) @ W2.T + b2
    A  = adj + I
    r  = rowsum(A) ** -0.5
    out = (r[:,None] * A * r[None,:]) @ h
        = r ⊙ (adj @ (r ⊙ h)) + r ⊙ (r ⊙ h)

Associativity: with h1 = x @ W1.T + b1 and g1 = [r ⊙ h1 | r] (257 cols),
    acc  = adj @ g1 + g1_own            # [rows, 257]  (spmm, K=8192)
    out  = r ⊙ (acc[:, :256] @ W2.T + acc[:, 256] ⊗ b2)

Distribution: row-shard adj and x across 8 cores (1024 rows each).
Per core:
  - stream the 32MB fp32 adj shard once; one ACT op per chunk does the
    bf16 cast + rowsum (accum_out); XBAR DMA-transposes (SBUF->SBUF, on
    the ACT HWDGE ring) land bf16 adjT [128, 64, 1024] resident in SBUF.
  - mm1 on the local x shard -> h1 (bf16, hid on partitions); AllGather
    h1 (1MB, overlaps the adj streaming).
  - AllGather rowsums (4KB); r = sqrt(1/(rowsum+1)).
  - g1 built by DMA-transposing gathered h1 + r scale.
  - spmm: per local i-tile accumulate 64 bf16 matmuls [128x128]@[128x257]
    into one PSUM bank; small @W2T tail + bias + r_i scale; DMA out.
"""

import sys

if "/opt/trn_rl_repo" not in sys.path:
    sys.path.insert(0, "/opt/trn_rl_repo")

import numpy as np

import concourse.bass as bass
import concourse.mybir as mybir
import concourse.tile as tile
from concourse import bacc
from concourse.bass_utils import run_bass_kernel_spmd
from concourse.masks import make_identity

F32 = mybir.dt.float32
BF16 = mybir.dt.bfloat16
P = 128
NCORES = 8


def build_nc(n_nodes=8192, in_f=512, hid=256, out_f=512, cw=2048, debug_dump=False):
    """Build and compile the per-core SPMD kernel."""
    cw = min(cw, n_nodes)
    SH = n_nodes // NCORES      # shard rows per core
    KB = SH // P                # row blocks in shard (= local i tiles)
    JT = n_nodes // P           # global j tiles
    QN = n_nodes // cw          # adj chunks per row block
    HALF = min(2 * cw, n_nodes)  # bf16 staging width (fewer, bigger tDMAs)
    NH = n_nodes // HALF        # staging buffers per row block
    QPH = HALF // cw            # read-chunks per staging buffer
    BPH = HALF // P             # 128-blocks per staging buffer
    FT = in_f // P              # input feature tiles
    HT = hid // P               # hidden tiles
    OF = out_f
    GW = hid + 1                # g1 logical width: [r*h1 | r]
    GWP = hid + 16              # padded block stride (32B-aligned for tDMA dests)
    IG = min(2, KB)             # i-stripes per mm1 group
    NG = IG * P                 # mm1 rhs free size

    nc = bacc.Bacc(
        "TRN2",
        target_bir_lowering=False,
        debug=False,
        num_devices=NCORES,
        dynamic_dma_scratch_size=4096,
    )
    adj_s = nc.dram_tensor("adj_shard", [SH, n_nodes], F32, kind="ExternalInput")
    x_s = nc.dram_tensor("x_shard", [SH, in_f], F32, kind="ExternalInput")
    W1 = nc.dram_tensor("W1", [hid, in_f], F32, kind="ExternalInput")
    b1 = nc.dram_tensor("b1", [hid], F32, kind="ExternalInput")
    W2 = nc.dram_tensor("W2", [out_f, hid], F32, kind="ExternalInput")
    b2 = nc.dram_tensor("b2", [out_f], F32, kind="ExternalInput")
    out = nc.dram_tensor("out_shard", [SH, OF], F32, kind="ExternalOutput")

    with tile.TileContext(nc) as tc:
        with (
            tc.tile_pool(name="const", bufs=1) as cpool,
            tc.tile_pool(name="big", bufs=1) as bigpool,
            tc.tile_pool(name="chbf", bufs=3) as chbf_pool,
            tc.tile_pool(name="xt", bufs=1) as xt_pool,
            tc.tile_pool(name="acc_bf", bufs=1) as accbf_pool,
            tc.tile_pool(name="vb", bufs=1) as vb_pool,
            tc.tile_pool(name="out_sb", bufs=1) as out_pool,
            tc.tile_pool(name="pmm", bufs=2, space="PSUM") as pmm_pool,
            tc.tile_pool(name="pacc", bufs=2, space="PSUM") as pacc_pool,
            tc.tile_pool(name="ptail", bufs=2, space="PSUM") as ptail_pool,
            tc.tile_pool(name="dram", bufs=1, space="DRAM") as dram,
        ):
            # ---------- constants ----------
            ones_bf = cpool.tile([1, max(OF, P)], BF16)
            nc.vector.memset(ones_bf, 1.0)
            ones_col = cpool.tile([P, 1], BF16)
            nc.vector.memset(ones_col, 1.0)
            # SWDGE DMA casts f32 -> bf16 in flight
            b1_bf = cpool.tile([1, hid], BF16)
            nc.gpsimd.dma_start(b1_bf, b1.ap()[None, :])
            b2_bf = cpool.tile([1, out_f], BF16)
            nc.gpsimd.dma_start(b2_bf, b2.ap()[None, :])
            # b2 broadcast to all partitions (for the v*b2 outer product)
            pb = pmm_pool.tile([P, OF], F32, tag="pmm")
            nc.tensor.matmul(pb, ones_bf[:1, :P], b2_bf[:1, :], start=True, stop=True)
            b2_bcast = cpool.tile([P, OF], BF16)
            nc.scalar.copy(b2_bcast, pb)

            # ---------- weights: W1T [P, FT, hid], W2T [P, HT, OF] via tDMA ----------
            W1T = cpool.tile([P, FT, hid], BF16)
            W2T = cpool.tile([P, HT, OF], BF16)
            for s in range(HT):  # W1 row stripes (hid)
                w_bf = chbf_pool.tile([P, HALF], BF16, tag="chbf")
                nc.gpsimd.dma_start(w_bf[:, :in_f], W1.ap()[s * P:(s + 1) * P, :])
                nc.scalar.dma_start_transpose(
                    W1T[:, :, s * P:(s + 1) * P], w_bf[:, :in_f]
                )
            for s in range(out_f // P):  # W2 row stripes (out_f)
                w_bf = chbf_pool.tile([P, HALF], BF16, tag="chbf")
                nc.gpsimd.dma_start(w_bf[:, :hid], W2.ap()[s * P:(s + 1) * P, :])
                nc.scalar.dma_start_transpose(
                    W2T[:, :, s * P:(s + 1) * P], w_bf[:, :hid]
                )

            # ---------- mm1: h1_c [P, HT, SH] bf16 (hid on partitions) ----------
            h1_c = bigpool.tile([P, HT, SH], BF16)
            for grp in range(SH // NG):
                xt = xt_pool.tile([P, FT, NG], BF16)
                for s in range(IG):
                    row0 = (grp * IG + s) * P
                    x_bf = chbf_pool.tile([P, HALF], BF16, tag="chbf")
                    nc.gpsimd.dma_start(x_bf[:, :in_f], x_s.ap()[row0:row0 + P, :])
                    nc.scalar.dma_start_transpose(
                        xt[:, :, s * P:(s + 1) * P], x_bf[:, :in_f]
                    )
                for ht in range(HT):
                    pm = pmm_pool.tile([P, OF], F32, tag="pmm")
                    pm1 = pm[:, :NG]
                    nc.tensor.matmul(
                        pm1, b1_bf[:1, ht * P:(ht + 1) * P], ones_bf[:1, :NG],
                        start=True, stop=False,
                    )
                    for ft in range(FT):
                        nc.tensor.matmul(
                            pm1,
                            W1T[:, ft, ht * P:(ht + 1) * P],
                            xt[:, ft, :],
                            start=False, stop=(ft == FT - 1),
                        )
                    nc.scalar.copy(h1_c[:, ht, grp * NG:(grp + 1) * NG], pm1)

            # ---------- AllGather h1 ----------
            h1d = dram.tile([HT, P, SH], BF16)
            for ht in range(HT):
                nc.sync.dma_start(h1d[ht], h1_c[:, ht, :])
            h1g = dram.tile([NCORES * HT, P, SH], BF16, addr_space="Shared")
            nc.gpsimd.collective_compute(
                "AllGather",
                mybir.AluOpType.bypass,
                replica_groups=[list(range(NCORES))],
                ins=[h1d.opt()],
                outs=[h1g.opt()],
            )

            # ---------- stream adj shard: fused cast+rowsum, tDMA into adjT ----------
            adjT = bigpool.tile([P, KB, JT, P], BF16)
            rowsum_c = cpool.tile([P, KB], F32)
            for k in range(KB):
                for h in range(NH):
                    chbf = chbf_pool.tile([P, HALF], BF16, tag="chbf")
                    # SWDGE cast-DMA: fp32 HBM -> bf16 SBUF in one hop
                    nc.gpsimd.dma_start(
                        chbf, adj_s.ap()[k * P:(k + 1) * P, h * HALF:(h + 1) * HALF]
                    )
                    nc.scalar.dma_start_transpose(
                        adjT[:, k, h * BPH:(h + 1) * BPH, :], chbf
                    )
                # rowsums on PE: psum[:, 0] += adjT[:, jt, kslice].T @ ones
                prs = pacc_pool.tile([P, 1], F32, tag="prs")
                for jt in range(JT):
                    nc.tensor.matmul(
                        prs,
                        adjT[:, k, jt, :],
                        ones_col,
                        start=(jt == 0),
                        stop=(jt == JT - 1),
                    )
                nc.vector.tensor_copy(rowsum_c[:, k:k + 1], prs)

            # ---------- AllGather rowsums; r vectors ----------
            rsd = dram.tile([P, KB], F32)
            nc.sync.dma_start(rsd, rowsum_c)
            rsg = dram.tile([NCORES * P, KB], F32, addr_space="Shared")
            nc.gpsimd.collective_compute(
                "AllGather",
                mybir.AluOpType.bypass,
                replica_groups=[list(range(NCORES))],
                ins=[rsd.opt()],
                outs=[rsg.opt()],
            )
            # r_sb [P, NCORES, KB]: r for global tile jt = (c, k) at [:, c, k]
            rs_t = cpool.tile([P, NCORES, KB], F32)
            nc.sync.dma_start(rs_t, rsg.rearrange("(c p) k -> p c k", p=P))
            r_sb = cpool.tile([P, NCORES, KB], F32)
            nc.vector.tensor_scalar_add(rs_t, rs_t, 1.0)
            nc.vector.reciprocal(rs_t, rs_t)
            nc.scalar.sqrt(r_sb, rs_t)
            # local r for own rows
            r_own = cpool.tile([P, KB], F32)
            ro_t = cpool.tile([P, KB], F32)
            nc.vector.tensor_scalar_add(ro_t, rowsum_c, 1.0)
            nc.vector.reciprocal(ro_t, ro_t)
            nc.scalar.sqrt(r_own, ro_t)

            # ---------- g1 [P, JT, GW] = [r ⊙ h1ᵀ | r] for all rows ----------
            g1 = bigpool.tile([P, JT, GWP], BF16)
            for c in range(NCORES):
                for ht in range(HT):
                    # dest[p, k, q] = h1g[c*HT+ht, q, k*P+p]
                    nc.scalar.dma_start_transpose(
                        g1[:, c * KB:(c + 1) * KB, ht * P:(ht + 1) * P],
                        h1g[c * HT + ht, :, :],
                    )
            for jt in range(JT):
                c, k = jt // KB, jt % KB
                rc = r_sb[:, c, k:k + 1]
                nc.vector.tensor_scalar_mul(g1[:, jt, :hid], g1[:, jt, :hid], rc)
                nc.vector.tensor_copy(g1[:, jt, hid:GW], rc)
            # own-row g1 from local h1 (identity term)
            g1own = bigpool.tile([P, KB, GWP], BF16)
            for ht in range(HT):
                nc.scalar.dma_start_transpose(
                    g1own[:, :, ht * P:(ht + 1) * P], h1_c[:, ht, :]
                )
            for k in range(KB):
                rc = r_own[:, k:k + 1]
                nc.vector.tensor_scalar_mul(g1own[:, k, :hid], g1own[:, k, :hid], rc)
                nc.vector.tensor_copy(g1own[:, k, hid:GW], rc)

            # ---------- debug dumps ----------
            if debug_dump:
                d_adjT = nc.dram_tensor(
                    "d_adjT", [P, KB, JT, P], BF16, kind="ExternalOutput"
                )
                nc.sync.dma_start(d_adjT.ap(), adjT)
                d_g1 = nc.dram_tensor("d_g1", [P, JT, GWP], BF16, kind="ExternalOutput")
                nc.sync.dma_start(d_g1.ap(), g1)
                d_h1 = nc.dram_tensor("d_h1", [P, HT, SH], BF16, kind="ExternalOutput")
                nc.sync.dma_start(d_h1.ap(), h1_c)
                d_rs = nc.dram_tensor("d_rs", [P, KB], F32, kind="ExternalOutput")
                nc.sync.dma_start(d_rs.ap(), rowsum_c)

            # ---------- spmm + tail ----------
            for it in range(KB):
                acc = pacc_pool.tile([P, GW], F32)
                for jt in range(JT):
                    nc.tensor.matmul(
                        acc,
                        adjT[:, it, jt, :],
                        g1[:, jt, :GW],
                        start=(jt == 0),
                        stop=(jt == JT - 1),
                    )
                # acc += g1_own (identity term), cast to bf16 for the tail
                nc.vector.tensor_tensor(acc, acc, g1own[:, it, :GW], mybir.AluOpType.add)
                accbf = accbf_pool.tile([P, hid], BF16)
                nc.scalar.copy(accbf, acc[:, :hid])
                vcol = accbf_pool.tile([P, 1], F32, tag="vcol")
                nc.vector.tensor_copy(vcol, acc[:, hid:GW])
                # accT [P(h1), HT, P(i)] via SBUF->SBUF tDMA
                accT = accbf_pool.tile([P, HT, P], BF16, tag="accT")
                nc.scalar.dma_start_transpose(accT, accbf)
                # tail: out = r_own ⊙ (accT.T @ W2T + v ⊗ b2)
                pt = ptail_pool.tile([P, OF], F32)
                for ht in range(HT):
                    nc.tensor.matmul(
                        pt, accT[:, ht, :], W2T[:, ht, :],
                        start=(ht == 0), stop=(ht == HT - 1),
                    )
                vb = vb_pool.tile([P, OF], BF16)
                nc.vector.tensor_scalar_mul(vb, b2_bcast, vcol)
                nc.vector.tensor_tensor(pt, pt, vb, mybir.AluOpType.add)
                o_sb = out_pool.tile([P, OF], F32)
                nc.vector.tensor_scalar_mul(o_sb, pt, r_own[:, it:it + 1])
                nc.sync.dma_start(out.ap()[it * P:(it + 1) * P, :], o_sb)

    nc.compile()
    return nc


_NC_CACHE = {}


def _get_nc(key=8192):
    if key not in _NC_CACHE:
        _NC_CACHE[key] = build_nc(n_nodes=key)
    return _NC_CACHE[key]


def kernel(x, adj, W1, b1, W2, b2):
    """Full-input entry point: shard internally across 8 cores, return full output."""
    n = adj.shape[0]
    nc = _get_nc(n)
    sh = n // NCORES
    x = np.ascontiguousarray(np.asarray(x, dtype=np.float32))
    adj = np.ascontiguousarray(np.asarray(adj, dtype=np.float32))
    W1 = np.ascontiguousarray(np.asarray(W1, dtype=np.float32))
    b1 = np.ascontiguousarray(np.asarray(b1, dtype=np.float32))
    W2 = np.ascontiguousarray(np.asarray(W2, dtype=np.float32))
    b2 = np.ascontiguousarray(np.asarray(b2, dtype=np.float32))
    in_maps = [
        {
            "adj_shard": adj[c * sh:(c + 1) * sh],
            "x_shard": x[c * sh:(c + 1) * sh],
            "W1": W1,
            "b1": b1,
            "W2": W2,
            "b2": b2,
        }
        for c in range(NCORES)
    ]
    res = run_bass_kernel_spmd(nc, in_maps, list(range(NCORES)), trace=False)
    return np.concatenate(
        [res.results[c]["out_shard"] for c in range(NCORES)], axis=0
    )


# revision 24
# speedup vs baseline: 3.0355x; 1.0908x over previous
"""GCN layer kernel for 8 Trainium2 NeuronCores.

Math (reference):
    h  = (x @ W1.T + b1) @ W2.T + b2
    A  = adj + I
    r  = rowsum(A) ** -0.5
    out = (r[:,None] * A * r[None,:]) @ h
        = r ⊙ (adj @ (r ⊙ h)) + r ⊙ (r ⊙ h)

Associativity: with h1 = x @ W1.T + b1 and g1 = [r ⊙ h1 | r] (257 cols),
    acc  = adj @ g1 + g1_own            # [rows, 257]  (spmm, K=8192)
    out  = r ⊙ (acc[:, :256] @ W2.T + acc[:, 256] ⊗ b2)

Distribution: row-shard adj and x across 8 cores (1024 rows each).
Per core:
  - stream the 32MB fp32 adj shard once; one ACT op per chunk does the
    bf16 cast + rowsum (accum_out); XBAR DMA-transposes (SBUF->SBUF, on
    the ACT HWDGE ring) land bf16 adjT [128, 64, 1024] resident in SBUF.
  - mm1 on the local x shard -> h1 (bf16, hid on partitions); AllGather
    h1 (1MB, overlaps the adj streaming).
  - AllGather rowsums (4KB); r = sqrt(1/(rowsum+1)).
  - g1 built by DMA-transposing gathered h1 + r scale.
  - spmm: per local i-tile accumulate 64 bf16 matmuls [128x128]@[128x257]
    into one PSUM bank; small @W2T tail + bias + r_i scale; DMA out.
"""

import sys

if "/opt/trn_rl_repo" not in sys.path:
    sys.path.insert(0, "/opt/trn_rl_repo")

import numpy as np

import concourse.bass as bass
import concourse.mybir as mybir
import concourse.tile as tile
from concourse import bacc
from concourse.bass_utils import run_bass_kernel_spmd
from concourse.masks import make_identity

F32 = mybir.dt.float32
BF16 = mybir.dt.bfloat16
P = 128
NCORES = 8


def build_nc(n_nodes=8192, in_f=512, hid=256, out_f=512, cw=2048, debug_dump=False):
    """Build and compile the per-core SPMD kernel."""
    cw = min(cw, n_nodes)
    SH = n_nodes // NCORES      # shard rows per core
    KB = SH // P                # row blocks in shard (= local i tiles)
    JT = n_nodes // P           # global j tiles
    QN = n_nodes // cw          # adj chunks per row block
    HALF = min(2 * cw, n_nodes)  # bf16 staging width (fewer, bigger tDMAs)
    NH = n_nodes // HALF        # staging buffers per row block
    QPH = HALF // cw            # read-chunks per staging buffer
    BPH = HALF // P             # 128-blocks per staging buffer
    FT = in_f // P              # input feature tiles
    HT = hid // P               # hidden tiles
    OF = out_f
    GW = hid + 1                # g1 logical width: [r*h1 | r]
    GWP = hid + 16              # padded block stride (32B-aligned for tDMA dests)
    IG = min(2, KB)             # i-stripes per mm1 group
    NG = IG * P                 # mm1 rhs free size

    nc = bacc.Bacc(
        "TRN2",
        target_bir_lowering=False,
        debug=False,
        num_devices=NCORES,
        dynamic_dma_scratch_size=4096,
    )
    adj_s = nc.dram_tensor("adj_shard", [SH, n_nodes], F32, kind="ExternalInput")
    x_s = nc.dram_tensor("x_shard", [SH, in_f], F32, kind="ExternalInput")
    W1 = nc.dram_tensor("W1", [hid, in_f], F32, kind="ExternalInput")
    b1 = nc.dram_tensor("b1", [hid], F32, kind="ExternalInput")
    W2 = nc.dram_tensor("W2", [out_f, hid], F32, kind="ExternalInput")
    b2 = nc.dram_tensor("b2", [out_f], F32, kind="ExternalInput")
    out = nc.dram_tensor("out_shard", [SH, OF], F32, kind="ExternalOutput")

    with tile.TileContext(nc) as tc:
        with (
            tc.tile_pool(name="const", bufs=1) as cpool,
            tc.tile_pool(name="big", bufs=1) as bigpool,
            tc.tile_pool(name="chbf", bufs=3) as chbf_pool,
            tc.tile_pool(name="xt", bufs=1) as xt_pool,
            tc.tile_pool(name="acc_bf", bufs=1) as accbf_pool,
            tc.tile_pool(name="vb", bufs=1) as vb_pool,
            tc.tile_pool(name="out_sb", bufs=1) as out_pool,
            tc.tile_pool(name="pmm", bufs=2, space="PSUM") as pmm_pool,
            tc.tile_pool(name="pacc", bufs=2, space="PSUM") as pacc_pool,
            tc.tile_pool(name="ptr", bufs=2, space="PSUM") as ptr_pool,
            tc.tile_pool(name="dram", bufs=1, space="DRAM") as dram,
        ):
            # ---------- constants ----------
            ones_bf = cpool.tile([1, max(OF, P)], BF16)
            nc.vector.memset(ones_bf, 1.0)
            ones_col = cpool.tile([P, 1], BF16)
            nc.vector.memset(ones_col, 1.0)
            ident_bf = cpool.tile([P, P], BF16)
            make_identity(nc, ident_bf)
            # SWDGE DMA casts f32 -> bf16 in flight
            b1_bf = cpool.tile([1, hid], BF16)
            nc.gpsimd.dma_start(b1_bf, b1.ap()[None, :])
            b2_bf = cpool.tile([1, out_f], BF16)
            nc.gpsimd.dma_start(b2_bf, b2.ap()[None, :])
            # b2 broadcast to all partitions (for the v*b2 outer product)
            pb = pmm_pool.tile([P, OF], F32, tag="pmm")
            nc.tensor.matmul(pb, ones_bf[:1, :P], b2_bf[:1, :], start=True, stop=True)
            b2_bcast = cpool.tile([P, OF], BF16)
            nc.scalar.copy(b2_bcast, pb)

            # ---------- weights: W1T [P, FT, hid], W2T [P, HT, OF] via tDMA ----------
            W1T = cpool.tile([P, FT, hid], BF16)
            W2T = cpool.tile([P, HT, OF], BF16)
            for s in range(HT):  # W1 row stripes (hid)
                w_bf = chbf_pool.tile([P, HALF], BF16, tag="chbf")
                nc.gpsimd.dma_start(w_bf[:, :in_f], W1.ap()[s * P:(s + 1) * P, :])
                nc.scalar.dma_start_transpose(
                    W1T[:, :, s * P:(s + 1) * P], w_bf[:, :in_f]
                )
            for s in range(out_f // P):  # W2 row stripes (out_f)
                w_bf = chbf_pool.tile([P, HALF], BF16, tag="chbf")
                nc.gpsimd.dma_start(w_bf[:, :hid], W2.ap()[s * P:(s + 1) * P, :])
                nc.scalar.dma_start_transpose(
                    W2T[:, :, s * P:(s + 1) * P], w_bf[:, :hid]
                )

            # ---------- mm1: h1_c [P, HT, SH] bf16 (hid on partitions) ----------
            h1_c = bigpool.tile([P, HT, SH], BF16)
            for grp in range(SH // NG):
                xt = xt_pool.tile([P, FT, NG], BF16)
                for s in range(IG):
                    row0 = (grp * IG + s) * P
                    x_bf = chbf_pool.tile([P, HALF], BF16, tag="chbf")
                    nc.gpsimd.dma_start(x_bf[:, :in_f], x_s.ap()[row0:row0 + P, :])
                    nc.scalar.dma_start_transpose(
                        xt[:, :, s * P:(s + 1) * P], x_bf[:, :in_f]
                    )
                for ht in range(HT):
                    pm = pmm_pool.tile([P, OF], F32, tag="pmm")
                    pm1 = pm[:, :NG]
                    nc.tensor.matmul(
                        pm1, b1_bf[:1, ht * P:(ht + 1) * P], ones_bf[:1, :NG],
                        start=True, stop=False,
                    )
                    for ft in range(FT):
                        nc.tensor.matmul(
                            pm1,
                            W1T[:, ft, ht * P:(ht + 1) * P],
                            xt[:, ft, :],
                            start=False, stop=(ft == FT - 1),
                        )
                    nc.scalar.copy(h1_c[:, ht, grp * NG:(grp + 1) * NG], pm1)

            # ---------- AllGather h1 ----------
            h1d = dram.tile([HT, P, SH], BF16)
            for ht in range(HT):
                nc.sync.dma_start(h1d[ht], h1_c[:, ht, :])
            h1g = dram.tile([NCORES * HT, P, SH], BF16, addr_space="Shared")
            nc.gpsimd.collective_compute(
                "AllGather",
                mybir.AluOpType.bypass,
                replica_groups=[list(range(NCORES))],
                ins=[h1d.opt()],
                outs=[h1g.opt()],
            )

            # ---------- stream adj shard: fused cast+rowsum, tDMA into adjT ----------
            adjT = bigpool.tile([P, KB, JT, P], BF16)
            rowsum_c = cpool.tile([P, KB], F32)
            for k in range(KB):
                partials = cpool.tile([P, NH], F32, tag="partials")
                pe_jts = []
                for h in range(NH):
                    chbf = chbf_pool.tile([P, HALF], BF16, tag="chbf")
                    # SWDGE cast-DMA: fp32 HBM -> bf16 SBUF in one hop
                    nc.gpsimd.dma_start(
                        chbf, adj_s.ap()[k * P:(k + 1) * P, h * HALF:(h + 1) * HALF]
                    )
                    if h % 2 == 0 or NH == 1:
                        # XBAR transpose half; rowsum via DVE reduce
                        nc.scalar.dma_start_transpose(
                            adjT[:, k, h * BPH:(h + 1) * BPH, :], chbf
                        )
                        nc.vector.reduce_sum(
                            partials[:, h:h + 1], chbf, axis=mybir.AxisListType.X
                        )
                    else:
                        # PE transpose half (4 blocks per PSUM tile; DVE copy out)
                        for g in range(BPH // 4):
                            pt4 = ptr_pool.tile([P, 4, P], BF16)
                            for b4 in range(4):
                                nc.tensor.transpose(
                                    pt4[:, b4, :],
                                    chbf[:, (g * 4 + b4) * P:(g * 4 + b4 + 1) * P],
                                    ident_bf,
                                )
                            nc.vector.tensor_copy(
                                adjT[:, k, h * BPH + g * 4:h * BPH + (g + 1) * 4, :],
                                pt4,
                            )
                        pe_jts.append(h)
                # rowsums for PE halves: psum += adjT[:, k, jt, :].T @ ones
                if pe_jts:
                    prs = pacc_pool.tile([P, 1], F32, tag="prs")
                    n_mm = len(pe_jts) * BPH
                    i_mm = 0
                    for h in pe_jts:
                        for b in range(BPH):
                            nc.tensor.matmul(
                                prs,
                                adjT[:, k, h * BPH + b, :],
                                ones_col,
                                start=(i_mm == 0),
                                stop=(i_mm == n_mm - 1),
                            )
                            i_mm += 1
                    nc.vector.tensor_copy(partials[:, pe_jts[0]:pe_jts[0] + 1], prs)
                nc.vector.reduce_sum(
                    rowsum_c[:, k:k + 1], partials, axis=mybir.AxisListType.X
                )

            # ---------- AllGather rowsums; r vectors ----------
            rsd = dram.tile([P, KB], F32)
            nc.sync.dma_start(rsd, rowsum_c)
            rsg = dram.tile([NCORES * P, KB], F32, addr_space="Shared")
            nc.gpsimd.collective_compute(
                "AllGather",
                mybir.AluOpType.bypass,
                replica_groups=[list(range(NCORES))],
                ins=[rsd.opt()],
                outs=[rsg.opt()],
            )
            # r_sb [P, NCORES, KB]: r for global tile jt = (c, k) at [:, c, k]
            rs_t = cpool.tile([P, NCORES, KB], F32)
            nc.sync.dma_start(rs_t, rsg.rearrange("(c p) k -> p c k", p=P))
            r_sb = cpool.tile([P, NCORES, KB], F32)
            nc.vector.tensor_scalar_add(rs_t, rs_t, 1.0)
            nc.vector.reciprocal(rs_t, rs_t)
            nc.scalar.sqrt(r_sb, rs_t)
            # local r for own rows
            r_own = cpool.tile([P, KB], F32)
            ro_t = cpool.tile([P, KB], F32)
            nc.vector.tensor_scalar_add(ro_t, rowsum_c, 1.0)
            nc.vector.reciprocal(ro_t, ro_t)
            nc.scalar.sqrt(r_own, ro_t)

            # ---------- g1 [P, JT, GW] = [r ⊙ h1ᵀ | r] for all rows ----------
            g1 = bigpool.tile([P, JT, GWP], BF16)
            for c in range(NCORES):
                for ht in range(HT):
                    # dest[p, k, q] = h1g[c*HT+ht, q, k*P+p]
                    nc.scalar.dma_start_transpose(
                        g1[:, c * KB:(c + 1) * KB, ht * P:(ht + 1) * P],
                        h1g[c * HT + ht, :, :],
                    )
            for jt in range(JT):
                c, k = jt // KB, jt % KB
                rc = r_sb[:, c, k:k + 1]
                nc.vector.tensor_scalar_mul(g1[:, jt, :hid], g1[:, jt, :hid], rc)
                nc.vector.tensor_copy(g1[:, jt, hid:GW], rc)
            # own-row g1 from local h1 (identity term)
            g1own = bigpool.tile([P, KB, GWP], BF16)
            for ht in range(HT):
                nc.scalar.dma_start_transpose(
                    g1own[:, :, ht * P:(ht + 1) * P], h1_c[:, ht, :]
                )
            for k in range(KB):
                rc = r_own[:, k:k + 1]
                nc.vector.tensor_scalar_mul(g1own[:, k, :hid], g1own[:, k, :hid], rc)
                nc.vector.tensor_copy(g1own[:, k, hid:GW], rc)

            # ---------- debug dumps ----------
            if debug_dump:
                d_adjT = nc.dram_tensor(
                    "d_adjT", [P, KB, JT, P], BF16, kind="ExternalOutput"
                )
                nc.sync.dma_start(d_adjT.ap(), adjT)
                d_g1 = nc.dram_tensor("d_g1", [P, JT, GWP], BF16, kind="ExternalOutput")
                nc.sync.dma_start(d_g1.ap(), g1)
                d_h1 = nc.dram_tensor("d_h1", [P, HT, SH], BF16, kind="ExternalOutput")
                nc.sync.dma_start(d_h1.ap(), h1_c)
                d_rs = nc.dram_tensor("d_rs", [P, KB], F32, kind="ExternalOutput")
                nc.sync.dma_start(d_rs.ap(), rowsum_c)

            # ---------- spmm + tail ----------
            for it in range(KB):
                acc = pacc_pool.tile([P, GW], F32)
                for jt in range(JT):
                    nc.tensor.matmul(
                        acc,
                        adjT[:, it, jt, :],
                        g1[:, jt, :GW],
                        start=(jt == 0),
                        stop=(jt == JT - 1),
                    )
                # acc += g1_own (identity term), cast to bf16 for the tail
                nc.vector.tensor_tensor(acc, acc, g1own[:, it, :GW], mybir.AluOpType.add)
                accbf = accbf_pool.tile([P, hid], BF16)
                nc.scalar.copy(accbf, acc[:, :hid])
                vcol = accbf_pool.tile([P, 1], F32, tag="vcol")
                nc.vector.tensor_copy(vcol, acc[:, hid:GW])
                # accT [P(h1), HT, P(i)] via SBUF->SBUF tDMA
                accT = accbf_pool.tile([P, HT, P], BF16, tag="accT")
                nc.scalar.dma_start_transpose(accT, accbf)
                # tail: out = r_own ⊙ (accT.T @ W2T + v ⊗ b2)
                pt = pmm_pool.tile([P, OF], F32, tag="pmm")
                for ht in range(HT):
                    nc.tensor.matmul(
                        pt, accT[:, ht, :], W2T[:, ht, :],
                        start=(ht == 0), stop=(ht == HT - 1),
                    )
                vb = vb_pool.tile([P, OF], BF16)
                nc.vector.tensor_scalar_mul(vb, b2_bcast, vcol)
                nc.vector.tensor_tensor(pt, pt, vb, mybir.AluOpType.add)
                o_sb = out_pool.tile([P, OF], F32)
                nc.vector.tensor_scalar_mul(o_sb, pt, r_own[:, it:it + 1])
                nc.sync.dma_start(out.ap()[it * P:(it + 1) * P, :], o_sb)

    nc.compile()
    return nc


_NC_CACHE = {}


def _get_nc(key=8192):
    if key not in _NC_CACHE:
        _NC_CACHE[key] = build_nc(n_nodes=key)
    return _NC_CACHE[key]


def kernel(x, adj, W1, b1, W2, b2):
    """Full-input entry point: shard internally across 8 cores, return full output."""
    n = adj.shape[0]
    nc = _get_nc(n)
    sh = n // NCORES
    x = np.ascontiguousarray(np.asarray(x, dtype=np.float32))
    adj = np.ascontiguousarray(np.asarray(adj, dtype=np.float32))
    W1 = np.ascontiguousarray(np.asarray(W1, dtype=np.float32))
    b1 = np.ascontiguousarray(np.asarray(b1, dtype=np.float32))
    W2 = np.ascontiguousarray(np.asarray(W2, dtype=np.float32))
    b2 = np.ascontiguousarray(np.asarray(b2, dtype=np.float32))
    in_maps = [
        {
            "adj_shard": adj[c * sh:(c + 1) * sh],
            "x_shard": x[c * sh:(c + 1) * sh],
            "W1": W1,
            "b1": b1,
            "W2": W2,
            "b2": b2,
        }
        for c in range(NCORES)
    ]
    res = run_bass_kernel_spmd(nc, in_maps, list(range(NCORES)), trace=False)
    return np.concatenate(
        [res.results[c]["out_shard"] for c in range(NCORES)], axis=0
    )
